# revision 13
# baseline (speedup 1.0000x reference)
"""CBOW negative-sampling loss on 8 Trainium2 NeuronCores — v7.

Measured constraints driving this design (HW microbenchmarks + traces):
  - per-core DMA ceiling ~330 GB/s (two HWDGE rings help <15%): fp8
    packing is mandatory (19MB/core -> ~58us floor; bf16 would be 117).
  - DVE is 2 elem/lane/cyc only for 2-byte dtypes; tensor_reduce is 1x.
  - DoubleRow fp8 matmuls pipeline at ~202ns per 480-col MM when PE
    stays fed (chained PSUM accumulation does not serialize the
    stream); DR requires PSUM dst partition offset 0.
  - Cross-engine chains (tail on ACT->Pool->DVE in v6) cause FIFO
    head-of-line blocking on the DMA rings and ACT queue; keeping the
    whole contraction on PE is cheaper than any split.

Pipeline per 128-sample tile, 8 groups of 16 samples:
  - PE: 16 DR matmuls accumulate the FULL 300-d dot products into one
    [128, 480] PSUM bank: 8x d0:256 ([128,2,480] moving) + 8x d256:300
    ([22,2,480] moving). Group q's stationary is full-width with the
    16 target vectors at columns 16q:16q+16, zeros elsewhere (DR needs
    dst partition 0; groups stack on partitions via accumulation, the
    zero columns add nothing). Cell [m, b*30+j] =
    tgt_m . row_{16*(m//16)+b}; diagonal b == m%16 is the logit.
  - ACT: scatter-copies of the (tile+2) targets into zero-padded
    A/B/C/D stationary buffers, and contiguous PSUM -> bf16 evac.
  - DVE: +maskconst (0 on diagonal, -1e30 off) then tensor_reduce(max)
    over b -> L[:, t, 30].
  - Post: sign flip, stable softplus (Exp/Ln on ACT), weighted
    accumulate -> [128,1]; host sums across cores /B.
"""

import sys

for _p in ("/opt/trn_rl_repo", "/opt/pypackages"):
    if _p not in sys.path:
        sys.path.append(_p)

import ml_dtypes
import numpy as np

import concourse.bass as bass
import concourse.bacc as bacc
import concourse.tile as tile
from concourse import mybir
from concourse.bass_utils import run_bass_kernel_spmd

V = 100000
D = 300
B = 16384
C = 10
K = 20
NCORES = 8
P = 128
NJ = C + K  # 30
BCORE = B // NCORES  # 2048
NT = BCORE // P  # 16
D2 = 22  # d 256:300 as [22, 2]
W1 = 2 * NJ * P  # 7680
NEG_INF = -1.0e30

F8NP = ml_dtypes.float8_e4m3
BFNP = ml_dtypes.bfloat16
F8 = mybir.dt.float8e4
BF = mybir.dt.bfloat16
_f32 = mybir.dt.float32
DR = mybir.MatmulPerfMode.DoubleRow


def _ap(sliced, dims):
    return bass.AP(sliced.tensor, sliced.offset, [sliced.ap[0], *dims])


def build_nc(nt: int):
    nc = bacc.Bacc(None, target_bir_lowering=False, debug=False)
    AF = mybir.ActivationFunctionType
    OP = mybir.AluOpType

    mv01 = nc.dram_tensor("mv01", [nt * P, W1], F8, kind="ExternalInput")
    mv2 = nc.dram_tensor("mv2", [nt * D2, W1], F8, kind="ExternalInput")
    st01d = nc.dram_tensor("st01", [P, nt * 256], F8, kind="ExternalInput")
    st2d = nc.dram_tensor("st2", [D2, nt * 256], F8, kind="ExternalInput")
    maskd = nc.dram_tensor("maskadd", [P, 16 * NJ], BF, kind="ExternalInput")
    sgnd = nc.dram_tensor("sgn", [P, NJ], BF, kind="ExternalInput")
    wzd = nc.dram_tensor("wz", [P, NJ], _f32, kind="ExternalInput")
    out = nc.dram_tensor("out", [P, 1], _f32, kind="ExternalOutput")

    with tile.TileContext(nc) as tc:
        with (
            tc.tile_pool(name="g1p", bufs=6) as g1p,
            tc.tile_pool(name="g2p", bufs=6) as g2p,
            tc.tile_pool(name="yp", bufs=4) as yp,
            tc.tile_pool(name="mp", bufs=4) as mp,
            tc.tile_pool(name="pp", bufs=6, space="PSUM") as pp,
            tc.tile_pool(name="singles", bufs=1) as singles,
        ):
            st01s = singles.tile([P, nt, 2, P], F8)
            nc.scalar.dma_start(out=st01s[:], in_=st01d[:])
            st2s = singles.tile([D2, nt, 2, P], F8)
            nc.scalar.dma_start(out=st2s[:], in_=st2d[:])
            mask = singles.tile([P, 16 * NJ], BF)
            nc.scalar.dma_start(out=mask[:], in_=maskd[:])
            sgn = singles.tile([P, NJ], BF)
            nc.scalar.dma_start(out=sgn[:], in_=sgnd[:])
            wz = singles.tile([P, NJ], _f32)
            nc.scalar.dma_start(out=wz[:], in_=wzd[:])
            # preload Exp/Ln activation tables during pipeline warmup
            warm = singles.tile([P, 1], _f32)
            nc.vector.memset(warm[:], 0.0)
            nc.scalar.activation(warm[:], warm[:], AF.Exp)
            nc.scalar.activation(warm[:], warm[:], AF.Ln, bias=1.0)

            # zero-padded stationaries [d, k, q, 128]: data blocks at
            # plane-q cols 16q:16q+16 ((q,128)-space stride 144)
            stb = []
            st2b = []
            for i in range(4):
                s_ = singles.tile([P, 2, 8, P], F8, tag=f"stb{i}")
                nc.vector.memset(s_[:].bitcast(_f32), 0.0)
                stb.append(s_)
                s2_ = singles.tile([D2, 2, 8, P], F8, tag=f"st2b{i}")
                nc.vector.memset(s2_[:].bitcast(_f32), 0.0)
                st2b.append(s2_)

            def scatter_st(t):
                nc.scalar.activation(
                    _ap(stb[t % 4][:, 0, 0, 0:16], [[1024, 2], [144, 8], [1, 16]]),
                    _ap(st01s[:, t, 0, 0:16], [[P, 2], [16, 8], [1, 16]]),
                    AF.Copy,
                )
                nc.scalar.activation(
                    _ap(st2b[t % 4][:, 0, 0, 0:16], [[1024, 2], [144, 8], [1, 16]]),
                    _ap(st2s[:, t, 0, 0:16], [[P, 2], [16, 8], [1, 16]]),
                    AF.Copy,
                )

            scatter_st(0)
            scatter_st(1)
            scatter_st(2)

            L = singles.tile([P, nt, NJ], BF)
            acc4 = singles.tile([P, 4], _f32)

            def post_chunk(ci):
                # softplus + weighted accumulate over tiles 4ci..4ci+3
                Ls = L[:, 4 * ci : 4 * (ci + 1), :]
                zc = singles.tile([P, 4, NJ], BF, tag=f"z{ci}")
                nc.vector.tensor_tensor(
                    out=zc[:], in0=Ls, in1=bc(sgn[:], 4), op=OP.mult
                )
                rlc = singles.tile([P, 4, NJ], BF, tag=f"rl{ci}")
                nc.vector.tensor_scalar_max(rlc[:], zc[:], 0.0)
                nac = singles.tile([P, 4, NJ], BF, tag=f"na{ci}")
                nc.vector.scalar_tensor_tensor(
                    out=nac[:], in0=zc[:], scalar=-1.0, in1=zc[:],
                    op0=OP.mult, op1=OP.min,
                )
                ec = singles.tile([P, 4, NJ], _f32, tag=f"e{ci}")
                nc.scalar.activation(ec[:], nac[:], AF.Exp)
                lc = singles.tile([P, 4, NJ], _f32, tag=f"l{ci}")
                nc.scalar.activation(lc[:], ec[:], AF.Ln, bias=1.0)
                spc = singles.tile([P, 4, NJ], _f32, tag=f"sp{ci}")
                nc.vector.tensor_tensor(
                    out=spc[:], in0=rlc[:], in1=lc[:], op=OP.add
                )
                spwc = singles.tile([P, 4, NJ], _f32, tag=f"spw{ci}")
                nc.vector.scalar_tensor_tensor(
                    out=spwc[:], in0=spc[:], scalar=1.0, in1=bc(wz[:], 4),
                    op0=OP.mult, op1=OP.mult,
                    accum_out=acc4[:, ci : ci + 1],
                )

            def bc(a, n):
                return bass.AP(a.tensor, a.offset, [a.ap[0], [0, n], a.ap[-1]])

            # pre-issue every tile's input DMAs: the WAR semaphores on
            # the ring pace them as pool buffers free up, and no DMA
            # issue ever queues behind compute on an engine sequencer
            g1s = []
            g2s = []
            for t in range(nt):
                g1 = g1p.tile([P, 2, NJ * P], F8, tag="g1")
                nc.sync.dma_start(
                    out=g1[:, 0, :], in_=mv01[t * P : (t + 1) * P, 0 : NJ * P]
                )
                nc.scalar.dma_start(
                    out=g1[:, 1, :],
                    in_=mv01[t * P : (t + 1) * P, NJ * P : W1],
                )
                g2 = g2p.tile([D2, 2, NJ * P], F8, tag="g2")
                nc.sync.dma_start(out=g2[:], in_=mv2[t * D2 : (t + 1) * D2])
                g1s.append(g1)
                g2s.append(g2)

            for t in range(nt):
                g1 = g1s[t]
                g2 = g2s[t]
                if t + 3 < nt:
                    scatter_st(t + 3)
                st = stb[t % 4]
                st2 = st2b[t % 4]

                ps = pp.tile([P, 512], _f32, tag="ps")
                o = ps[:, 0 : 16 * NJ]
                for q in range(8):
                    c0 = 480 * q
                    nc.tensor.matmul(
                        o,
                        st[:, :, q, :],
                        g1[:, :, c0 : c0 + 480],
                        start=(q == 0),
                        stop=False,
                        perf_mode=DR,
                    )
                for q in range(8):
                    c0 = 480 * q
                    nc.tensor.matmul(
                        o,
                        st2[:, :, q, :],
                        g2[:, :, c0 : c0 + 480],
                        start=False,
                        stop=(q == 7),
                        perf_mode=DR,
                    )

                # ACT: contiguous evac psum -> Y bf16 [P, (b, j)]
                Y = yp.tile([P, 16 * NJ], BF, tag="Y")
                nc.scalar.activation(Y[:], ps[:, 0 : 16 * NJ], AF.Copy)

                # DVE: masked max-extraction of the diagonal (b == p%16)
                M = mp.tile([P, 16 * NJ], BF, tag="M")
                nc.vector.tensor_tensor(
                    out=M[:], in0=Y[:], in1=mask[:], op=OP.add
                )
                nc.vector.tensor_reduce(
                    out=L[:, t, :],
                    in_=_ap(M[:], [[1, NJ], [NJ, 16]]),
                    axis=mybir.AxisListType.X,
                    op=OP.max,
                )
            for ci in range(4):
                post_chunk(ci)

            # final: acc = sum of the 4 chunk accumulators
            acc = singles.tile([P, 1], _f32)
            nc.vector.tensor_reduce(
                out=acc[:],
                in_=acc4[:],
                axis=mybir.AxisListType.X,
                op=OP.add,
            )
            nc.sync.dma_start(out=out[:], in_=acc[:])

    nc.compile()
    return nc


_NC_CACHE: dict = {}


def _get_nc(nt: int):
    if nt not in _NC_CACHE:
        _NC_CACHE[nt] = build_nc(nt)
    return _NC_CACHE[nt]


def kernel(i_emb, o_emb, context, target, neg_samples, _trace=False, _trace_kwargs=None):
    i_emb = np.asarray(i_emb, dtype=np.float32)
    o_emb = np.asarray(o_emb, dtype=np.float32)
    context = np.asarray(context).astype(np.int64)
    target = np.asarray(target).astype(np.int64)
    neg_samples = np.asarray(neg_samples).astype(np.int64)

    o8 = o_emb.astype(F8NP)
    i8 = i_emb.astype(F8NP)

    allj = np.concatenate([context, neg_samples], axis=1)  # [B, 30]
    rows = o8[allj]  # [B, 30, 300]
    tg = i8[target]  # [B, 300]

    r5 = rows.reshape(NCORES, NT, P, NJ, D)
    # mv01[c, t, d, k, s, j] = r5[c, t, s, j, 128k + d]
    m1 = r5[..., : 2 * P].reshape(NCORES, NT, P, NJ, 2, P)
    mv01 = np.ascontiguousarray(m1.transpose(0, 1, 5, 4, 2, 3)).reshape(
        NCORES, NT * P, W1
    )
    # mv2[c, t, d2, k2, s, j] = r5[c, t, s, j, 256 + 22*k2 + d2]
    m2 = r5[..., 2 * P : D].reshape(NCORES, NT, P, NJ, 2, D2)
    mv2 = np.ascontiguousarray(m2.transpose(0, 1, 5, 4, 2, 3)).reshape(
        NCORES, NT * D2, W1
    )

    t4 = tg.reshape(NCORES, NT, P, D)
    # st01[c, d, (t, k, s)] = tg[c, t, s, 128k + d]
    s1 = t4[..., : 2 * P].reshape(NCORES, NT, P, 2, P)
    st01 = np.ascontiguousarray(s1.transpose(0, 4, 1, 3, 2)).reshape(
        NCORES, P, NT * 256
    )
    # st2[c, d2, (t, k2, s)] = tg[c, t, s, 256 + 22*k2 + d2]
    s2 = t4[..., 2 * P : D].reshape(NCORES, NT, P, 2, D2)
    st2 = np.ascontiguousarray(s2.transpose(0, 4, 1, 3, 2)).reshape(
        NCORES, D2, NT * 256
    )

    # mask in (b, j) layout: 0 where b == p%16 else -inf
    pidx = np.arange(P)[:, None, None]
    bb = np.arange(16)[None, :, None]
    mrow = np.where((pidx % 16) == bb, 0.0, NEG_INF)  # [P, 16, 1]
    maskadd = np.ascontiguousarray(
        np.broadcast_to(mrow, (P, 16, NJ)).astype(BFNP)
    ).reshape(P, 16 * NJ)

    jj = np.arange(NJ)
    sgn_row = np.where(jj < C, -1.0, 1.0).astype(BFNP)
    wz_row = np.where(jj < C, 1.0 / C, 1.0).astype(np.float32)
    consts = {
        "maskadd": maskadd,
        "sgn": np.tile(sgn_row, (P, 1)),
        "wz": np.tile(wz_row, (P, 1)),
    }

    nc = _get_nc(NT)

    in_maps = []
    for c in range(NCORES):
        in_maps.append(
            {
                "mv01": mv01[c],
                "mv2": mv2[c],
                "st01": st01[c],
                "st2": st2[c],
                **consts,
            }
        )

    kw = {}
    if _trace:
        kw["trace"] = True
        if _trace_kwargs:
            kw.update(_trace_kwargs)
    res = run_bass_kernel_spmd(nc, in_maps, core_ids=list(range(NCORES)), **kw)

    total = np.float64(0.0)
    for c in range(NCORES):
        total += np.asarray(res.results[c]["out"], dtype=np.float64).sum()
    loss = np.float32(total / B)
    if _trace:
        return loss, res
    return loss
